# revision 9
# baseline (speedup 1.0000x reference)
"""ColBERT MaxSim kernel for 8 Trainium2 NeuronCores (Bass/Tile).

Strategy: data-parallel over the 256-doc batch (32 docs per core).

Host side:
  - compacts each doc's VALID tokens (d_mask is ~50% dense) to the front
    and pads to a fixed budget LT (= max valid count rounded up to 32)
    with a COPY of the doc's first valid token.  Duplicating a valid
    token leaves the per-(query,doc) max unchanged, so this is exactly
    equivalent to the reference's -inf masking.
  - computes the query side entirely on host in fp32:
    qn = l2norm(W @ q_h) [dim, 128q] is shipped as a bf16 constant.
  - pre-transposes to h-major layout, casts the doc stream to fp8(e4m3);
    W is pre-scaled by 8 so entries land in e4m3's normal range.  The
    scale cancels exactly in sim_raw * rsqrt(|8*W@d|^2).

Per core (32 docs = 8 quads; docs 8k..8k+8 share one 32-query batch):
  per pair of docs (DoubleRow fp8, K=256 per pass):
    pd[:, j] = W8.T @ dT[d]           [128dim, LT] f32 PSUM
    sq = pd^2   (one ACT square per pair, bf16 -> SBUF)
    db = bf16(pd)  (one DVE copy per pair -> SBUF)
  per quad g (4 docs on col-groups cg = d%4; M=32 matmuls run
  CONCURRENTLY in distinct 32-col groups of the PE array):
    sim[32cg:+32, :] = qn[:, qb].T @ db[d]        (raw scores, K=128)
    ssq[32cg:+32, :] = ones[:, :32].T @ sq[d]     (per-token sumsq)
    invb = rsqrt(ssq + eps)           (ACT)
    scaled = sim * invb               (DVE, bf16)
    maxcol[:, g] = max_tok(scaled)    (DVE reduce_max)
  out[4, 8] = blockones.T @ maxcol    (sum over 32 queries via matmul)

PE warm-up: the HAM clock gate keeps the PE at 1.2 GHz until ~3.4us of
sustained activity; dummy fp8 matmuls run while the first doc slab is
still in flight so the real work starts at 2.4 GHz.
"""

import numpy as np
import ml_dtypes

import concourse.bass as bass
import concourse.bacc as bacc
import concourse.mybir as mybir
import concourse.tile as tile
from concourse.bass_utils import run_bass_kernel_spmd

N_CORES = 8
H, HC, P = 768, 6, 128   # hidden dim, h-chunks, partitions
DIM = 128                # projection dim
DPC = 32                 # docs per core
QPC = 128                # query vectors per core (4 batches x 32)
PPQ = 8                  # passages per query
BF16 = mybir.dt.bfloat16
FP8 = mybir.dt.float8e4
F32 = mybir.dt.float32
EPS2 = 1e-12
LT_MIN = 128             # floor on compacted token budget
W8SCALE = 8.0            # fp8 pre-scale on W; cancels in normalization
N_WARMUP = 40            # dummy PE matmuls to lift the HAM clock gate

_LT = 288
_NC_CACHE = {}


def _rsqrt_act(nc, out, in_, bias_ap):
    """out = 1/sqrt(in_ + bias). Emits the Rsqrt activation directly
    (bass's helper refuses it; the 40k-entry reciprocal_sqrt HW table is
    plenty accurate for this kernel's fp8-dominated error budget)."""
    eng = nc.scalar
    ins = [eng.lower_ap(in_), eng.lower_ap(bias_ap),
           mybir.ImmediateValue(dtype=mybir.dt.float32, value=1.0),
           mybir.ImmediateValue(dtype=mybir.dt.float32, value=0.0)]
    return eng.add_instruction(mybir.InstActivation(
        name=nc.get_next_instruction_name(),
        func=mybir.ActivationFunctionType.Rsqrt,
        ins=ins, outs=[eng.lower_ap(out)]))


def _build_nc(lt):
    nc = bacc.Bacc()
    dt_d = nc.declare_dram_parameter(
        "dt", [DPC // 4, P, 4, HC, lt], FP8, isOutput=False)
    qn_d = nc.declare_dram_parameter("qn", [DIM, QPC], BF16, isOutput=False)
    wt8_d = nc.declare_dram_parameter("wt8", [P, HC, DIM], FP8, isOutput=False)
    out_d = nc.declare_dram_parameter("out", [4, DPC // 4], F32, isOutput=True)
    DR = mybir.MatmulPerfMode.DoubleRow

    with tile.TileContext(nc) as tc:
        with (
            tc.tile_pool(name="const", bufs=1) as const,
            tc.tile_pool(name="slab", bufs=4) as slabp,
            tc.tile_pool(name="work", bufs=2) as work,
            tc.tile_pool(name="psum", bufs=2, space=bass.MemorySpace.PSUM) as psum,
        ):
            # ---- input DMAs, ordered for earliest PE start ----
            wt8_s = const.tile([P, HC, DIM], FP8)
            nc.sync.dma_start(out=wt8_s, in_=wt8_d[:])
            slabs = {}
            slab0 = slabp.tile([P, 4, HC, lt], FP8, tag="slab")
            slabs[0] = slab0
            nc.sync.dma_start(out=slab0[:, 0], in_=dt_d[0, :, 0])
            nc.sync.dma_start(out=slab0[:, 1], in_=dt_d[0, :, 1])
            qn_s = const.tile([DIM, QPC], BF16)
            nc.sync.dma_start(out=qn_s, in_=qn_d[:])
            nc.sync.dma_start(out=slab0[:, 2], in_=dt_d[0, :, 2])
            nc.sync.dma_start(out=slab0[:, 3], in_=dt_d[0, :, 3])

            # ---- constants ----
            ones_raw = const.tile([P, 32], BF16)
            nc.vector.memset(ones_raw, 1.0)
            ones_s = const.tile([P, 32], BF16)     # all-ones lhsT
            nc.scalar.copy(ones_s, ones_raw)
            blk_raw = const.tile([P, 4], F32)      # block-diag ones: col b = 1
            nc.vector.memset(blk_raw, 0.0)         # on partitions 32b..32b+32
            for b in range(4):
                nc.vector.memset(blk_raw[32 * b:32 * b + 32, b:b + 1], 1.0)
            blockones = const.tile([P, 4], F32)
            nc.scalar.copy(blockones, blk_raw)
            eps_t = const.tile([P, 1], F32)        # rsqrt bias (l2norm eps^2)
            nc.vector.memset(eps_t, EPS2)
            maxcol = const.tile([P, DPC // 4], F32)   # [4docs x 32q, quads]

            # ---- PE warm-up while the first slab is in flight ----
            warm = psum.tile([P, 512], F32, tag="ssq")
            for i in range(N_WARMUP):
                nc.tensor.matmul(warm[:, :64], wt8_s[:, 0, :],
                                 wt8_s[:, i % HC, :64], start=True, stop=True)

            state = {}

            def emit_epi(g):
                sq4, db4 = state[g]
                qb = g // 2
                sim = psum.tile([P, 512], F32, tag="sim")
                for d in range(4):
                    nc.tensor.matmul(sim[32 * d:32 * d + 32, :lt],
                                     qn_s[:, 32 * qb:32 * qb + 32],
                                     db4[:, d, :],
                                     start=True, stop=True,
                                     tile_position=(0, 32 * d))
                ssq = psum.tile([P, 512], F32, tag="ssq")
                for d in range(4):
                    nc.tensor.matmul(ssq[32 * d:32 * d + 32, :lt],
                                     ones_s, sq4[:, d, :],
                                     start=True, stop=True,
                                     tile_position=(0, 32 * d))
                invb = work.tile([P, lt], F32, tag="invb")
                _rsqrt_act(nc, invb, ssq[:, :lt], eps_t[:, :])
                scaled = work.tile([P, lt], BF16, tag="scaled")
                nc.vector.tensor_mul(scaled, sim[:, :lt], invb)
                nc.vector.reduce_max(out=maxcol[:, g:g + 1], in_=scaled,
                                     axis=mybir.AxisListType.X)

            # ---- doc loop: 16 pairs, epilogue per quad, 1-pair pipelined ----
            for pp in range(DPC // 2):
                g = pp // 2
                if pp % 2 == 0:
                    if g > 0:
                        slab_g = slabp.tile([P, 4, HC, lt], FP8, tag="slab")
                        slabs[g] = slab_g
                        if g == 1:
                            for d in range(4):
                                nc.sync.dma_start(out=slab_g[:, d],
                                                  in_=dt_d[g, :, d])
                        else:
                            nc.sync.dma_start(out=slab_g, in_=dt_d[g])
                    sq4 = work.tile([P, 4, lt], BF16, tag="sq4", bufs=3)
                    db4 = work.tile([P, 4, lt], BF16, tag="db4", bufs=3)
                    state[g] = (sq4, db4)
                slab = slabs[g]
                sq4, db4 = state[g]
                pd = psum.tile([DIM, 2, 512], F32, tag="pd")
                for c in range(0, HC, 2):
                    for j in range(2):
                        d = 2 * (pp % 2) + j
                        nc.tensor.matmul(pd[:, j, :lt], wt8_s[:, c:c + 2, :],
                                         slab[:, d, c:c + 2, :],
                                         start=(c == 0), stop=(c == HC - 2),
                                         perf_mode=DR)
                pr = pp % 2
                # ACT drains PSUM (the only pd reader); DVE squares the bf16
                # copy at 2x 16-bit rate from SBUF
                dbp = db4[:, 2 * pr:2 * pr + 2, :]
                nc.scalar.copy(dbp, pd[:, :, :lt])
                nc.vector.tensor_mul(sq4[:, 2 * pr:2 * pr + 2, :], dbp, dbp)
                if pp % 2 == 0 and pp >= 4:
                    emit_epi(g - 2)
            emit_epi(DPC // 4 - 2)
            emit_epi(DPC // 4 - 1)

            # ---- sum over queries + writeback ----
            po = psum.tile([4, DPC // 4], F32, tag="sim")
            nc.tensor.matmul(po, blockones, maxcol, start=True, stop=True)
            out_s = work.tile([4, DPC // 4], F32, tag="outrow", bufs=1)
            nc.vector.tensor_copy(out_s, po)
            nc.sync.dma_start(out=out_d[:], in_=out_s)
    nc.compile()
    return nc


def _get_nc():
    nc = _NC_CACHE.get(_LT)
    if nc is None:
        nc = _NC_CACHE[_LT] = _build_nc(_LT)
    return nc


def _prep_in_maps(q_hidden, d_hidden, W, d_mask):
    global _LT
    bf16 = ml_dtypes.bfloat16
    f8 = ml_dtypes.float8_e4m3
    cnt = d_mask.sum(1)
    lt = int(max(LT_MIN, (int(cnt.max()) + 31) // 32 * 32))
    _LT = lt
    # compact valid tokens to the front; pad with the first valid token
    order = np.argsort(~d_mask, axis=1, kind="stable")
    idx = np.where(np.arange(lt)[None, :] >= cnt[:, None],
                   order[:, :1], order[:, :lt])
    d8 = np.take_along_axis(d_hidden, idx[:, :, None], axis=1).astype(f8)
    wt_t = np.ascontiguousarray(W.T.reshape(HC, P, DIM).transpose(1, 0, 2))
    wt8 = (wt_t * W8SCALE).astype(f8)
    # query side entirely on host: qn = l2norm(W @ q)  [dim, 128q] per core
    qf = q_hidden.reshape(-1, H).astype(np.float32)          # [256q, H]
    qp = qf @ W.T                                            # [256q, dim]
    qp /= np.maximum(np.sqrt((qp * qp).sum(-1, keepdims=True)), 1e-12)
    in_maps = []
    for c in range(N_CORES):
        dsl = d8[c * DPC:(c + 1) * DPC]                       # [32, lt, 768]
        dt = dsl.transpose(0, 2, 1).reshape(DPC // 4, 4, HC, P, lt)
        dt = np.ascontiguousarray(dt.transpose(0, 3, 1, 2, 4))  # [8,128,4,6,lt]
        qn = np.ascontiguousarray(qp[c * QPC:(c + 1) * QPC].T).astype(bf16)
        in_maps.append({"dt": dt, "qn": qn, "wt8": wt8})
    return in_maps


def _run(in_maps, trace=False, **kw):
    res = run_bass_kernel_spmd(
        _get_nc(), in_maps, core_ids=list(range(N_CORES)), trace=trace, **kw)
    # per-core output is [4, DPC//4] with doc = 4*col + row
    out = np.concatenate(
        [res.results[i]["out"].T.reshape(-1) for i in range(N_CORES)])
    return out.astype(np.float32), res


def kernel(q_hidden, d_hidden, W, d_mask, ppq):
    q_hidden = np.asarray(q_hidden, dtype=np.float32)
    d_hidden = np.asarray(d_hidden, dtype=np.float32)
    W = np.asarray(W, dtype=np.float32)
    d_mask = np.asarray(d_mask).astype(bool)
    in_maps = _prep_in_maps(q_hidden, d_hidden, W, d_mask)
    out, _ = _run(in_maps, trace=False)
    return out
